# revision 2
# baseline (speedup 1.0000x reference)
"""Bass/Tile kernel for nn_AttentionModel (B=32, S=2048, H=1024) on 8 TRN2 NeuronCores.

Math: the reference computes
    energy[b,s] = v . (W_h @ h_b + W_e @ e_bs + b_attn)
    attns       = softmax_s(energy)[:, None, :]
Everything downstream of the projection is a dot with v, so
    energy[b,s] = (W_e^T v) . e_bs + c_b
where c_b depends only on b. Softmax along s is shift-invariant, so c_b (the
rnn_hidden and b_attn terms) drops out exactly. The kernel therefore computes
    u = W_e^T v                     (tiny matvec, on TensorE)
    energy = E @ u                  (bandwidth-bound streaming dot, on VectorE)
    out = softmax_s(energy)         (ScalarE exp + VectorE reductions)
sharded data-parallel over batch: 4 batches per core, W_e/v replicated.
"""

import numpy as np

B, S, H = 32, 2048, 1024
NCORES = 8
BL = B // NCORES          # batches per core
R = BL * S                # 8192 encoder rows per core
P = 128                   # SBUF partitions
T = R // P                # 64 row-tiles per core
D = H
HC = H // P               # 8 contraction chunks for u = W_e^T v
G = 4                     # row-tiles per DMA chunk (G*512KB per dma_start)

_PROFILE = False          # test harness sets kernel._PROFILE = True for NTFF tracing
_cache = {}
last_results = None


def _build():
    import concourse.tile as tile
    from concourse import bacc, mybir

    f32 = mybir.dt.float32
    nc = bacc.Bacc("TRN2", target_bir_lowering=False, debug=False, num_devices=NCORES)
    e = nc.dram_tensor("e", [R, D], f32, kind="ExternalInput")
    w = nc.dram_tensor("w", [H, D], f32, kind="ExternalInput")
    v = nc.dram_tensor("v", [H], f32, kind="ExternalInput")
    out = nc.dram_tensor("out", [BL, S], f32, kind="ExternalOutput")
    scratch = nc.dram_tensor("scratch", [R], f32, kind="Internal")

    with tile.TileContext(nc) as tc:
        with (
            tc.tile_pool(name="consts", bufs=1) as consts,
            tc.tile_pool(name="wpool", bufs=1) as wpool,
            tc.tile_pool(name="chunks", bufs=3) as chunks,
            tc.tile_pool(name="prods", bufs=4) as prods,
            tc.tile_pool(name="psum", bufs=2, space="PSUM") as psum,
            tc.tile_pool(name="smax", bufs=1) as smax,
        ):
            # ---- u = W_e^T v on TensorE (contraction over h in 128-chunks) ----
            v_sb = consts.tile([P, HC], f32)
            nc.sync.dma_start(out=v_sb, in_=v.ap().rearrange("(c p) -> p c", p=P))
            w_sb = wpool.tile([P, HC, D], f32)
            nc.sync.dma_start(out=w_sb, in_=w.ap().rearrange("(c p) d -> p c d", p=P))
            u_sb = consts.tile([1, D], f32)
            for half in range(2):
                pu = psum.tile([1, 512], f32)
                for c in range(HC):
                    nc.tensor.matmul(
                        pu,
                        v_sb[:, c : c + 1],
                        w_sb[:, c, 512 * half : 512 * (half + 1)],
                        start=(c == 0),
                        stop=(c == HC - 1),
                    )
                nc.vector.tensor_copy(out=u_sb[:, 512 * half : 512 * (half + 1)], in_=pu)
            u_bc = consts.tile([P, D], f32)
            nc.gpsimd.partition_broadcast(u_bc, u_sb)

            # ---- energy[r] = e[r, :] . u, fused multiply+reduce on VectorE ----
            # Row r = p*T + t lives in partition p, tile t, so the [P, T] energy
            # tile maps linearly onto the flat [R] scratch layout.
            nrg = consts.tile([P, T], f32)
            e_r = e.ap().rearrange("(p t) d -> p t d", p=P)
            for t0 in range(0, T, G):
                ch = chunks.tile([P, G, D], f32)
                nc.sync.dma_start(out=ch, in_=e_r[:, t0 : t0 + G, :])
                for g in range(G):
                    # out = (e_tile * 1.0) * u ; accum_out = row-sum(out) = e_row . u
                    # (tensor_tensor_reduce is broken on this runtime; this
                    # InstTensorScalarPtr form is the working fused dot.)
                    pr = prods.tile([P, D], f32)
                    nc.vector.scalar_tensor_tensor(
                        out=pr,
                        in0=ch[:, g, :],
                        scalar=1.0,
                        in1=u_bc,
                        op0=mybir.AluOpType.mult,
                        op1=mybir.AluOpType.mult,
                        accum_out=nrg[:, t0 + g : t0 + g + 1],
                    )

            # ---- regroup energies to one batch row per partition ----
            nc.sync.dma_start(out=scratch.ap().rearrange("(p t) -> p t", p=P), in_=nrg)
            rows = smax.tile([BL, S], f32)
            nc.sync.dma_start(out=rows, in_=scratch.ap().rearrange("(b s) -> b s", b=BL))

            # ---- softmax along s ----
            nmax = smax.tile([BL, 1], f32)
            nc.vector.tensor_reduce(
                out=nmax, in_=rows, axis=mybir.AxisListType.X,
                op=mybir.AluOpType.max, negate=True,
            )
            prob = smax.tile([BL, S], f32)
            sums = smax.tile([BL, 1], f32)
            nc.scalar.activation(
                out=prob, in_=rows, func=mybir.ActivationFunctionType.Exp,
                bias=nmax, scale=1.0, accum_out=sums,
            )
            rec = smax.tile([BL, 1], f32)
            nc.vector.reciprocal(out=rec, in_=sums)
            res = smax.tile([BL, S], f32)
            nc.vector.tensor_scalar_mul(out=res, in0=prob, scalar1=rec)
            nc.sync.dma_start(out=out.ap(), in_=res)

    nc.compile()
    return nc


def kernel(encoder_outputs, rnn_hidden, W_attn, b_attn, v):
    global last_results
    from concourse.bass_utils import run_bass_kernel_spmd

    if "nc" not in _cache:
        _cache["nc"] = _build()
    nc = _cache["nc"]

    encoder_outputs = np.asarray(encoder_outputs, dtype=np.float32)
    w_e = np.ascontiguousarray(np.asarray(W_attn, dtype=np.float32)[:, H:])
    v_np = np.ascontiguousarray(np.asarray(v, dtype=np.float32))

    in_maps = []
    for c in range(NCORES):
        e_c = np.ascontiguousarray(
            encoder_outputs[c * BL : (c + 1) * BL].reshape(R, D)
        )
        in_maps.append({"e": e_c, "w": w_e, "v": v_np})

    last_results = run_bass_kernel_spmd(
        nc, in_maps, core_ids=list(range(NCORES)), trace=_PROFILE
    )
    outs = [last_results.results[c]["out"] for c in range(NCORES)]
    return np.concatenate(outs, axis=0).reshape(B, 1, S)


# revision 4
# speedup vs baseline: 1.2621x; 1.2621x over previous
"""Bass/Tile kernel for nn_AttentionModel (B=32, S=2048, H=1024) on 8 TRN2 NeuronCores.

Math: the reference computes
    energy[b,s] = v . (W_h @ h_b + W_e @ e_bs + b_attn)
    attns       = softmax_s(energy)[:, None, :]
Everything downstream of the projection is a dot with v, so
    energy[b,s] = (W_e^T v) . e_bs + c_b
where c_b depends only on b. Softmax along s is shift-invariant, so c_b (the
rnn_hidden and b_attn terms) drops out exactly. The kernel therefore computes
    u = W_e^T v                     (tiny matvec, TensorE, pipelined with W DMA)
    energy = E @ u                  (bandwidth-bound fused mult+reduce on VectorE)
    out = softmax_s(energy)         (per-batch, in SBUF via partition_all_reduce)
sharded data-parallel over batch: 4 batches per core, W_e/v replicated.

Per-core row mapping: local row r = b*S + p*TB + t  (p = SBUF partition,
t = row-tile index within batch, TB = S/128 = 16), so each batch's energies
land in one [128, TB] tile and its softmax/output never leave SBUF.
"""

import numpy as np

B, S, H = 32, 2048, 1024
NCORES = 8
BL = B // NCORES          # batches per core
P = 128                   # SBUF partitions
TB = S // P               # 16 row-tiles per batch
D = H
HC = H // P               # 8 contraction chunks for u = W_e^T v
G = 4                     # row-tiles per DMA chunk (G*512KB per dma_start)

_PROFILE = False          # test harness sets kernel._PROFILE = True for NTFF tracing
_cache = {}
last_results = None


def _build():
    import concourse.tile as tile
    from concourse import bacc, mybir
    from concourse.bass_isa import ReduceOp

    f32 = mybir.dt.float32
    Alu = mybir.AluOpType
    nc = bacc.Bacc("TRN2", target_bir_lowering=False, debug=False, num_devices=NCORES)
    e = nc.dram_tensor("e", [BL * S, D], f32, kind="ExternalInput")
    w = nc.dram_tensor("w", [H, D], f32, kind="ExternalInput")
    v = nc.dram_tensor("v", [H], f32, kind="ExternalInput")
    out = nc.dram_tensor("out", [BL, S], f32, kind="ExternalOutput")

    with tile.TileContext(nc) as tc:
        with (
            tc.tile_pool(name="consts", bufs=1) as consts,
            tc.tile_pool(name="wpool", bufs=HC) as wpool,
            tc.tile_pool(name="chunks", bufs=6) as chunks,
            tc.tile_pool(name="prods", bufs=4) as prods,
            tc.tile_pool(name="nrgs", bufs=2) as nrgs,
            tc.tile_pool(name="psum", bufs=2, space="PSUM") as psum,
            tc.tile_pool(name="smax", bufs=2) as smax,
        ):
            # Warm the ACT exp table while DMAs stream (first Exp otherwise
            # pays a ~2.7us table load in the softmax tail).
            warm = consts.tile([1, 1], f32)
            nc.vector.memset(warm, 0.0)
            nc.scalar.activation(
                out=warm, in_=warm, func=mybir.ActivationFunctionType.Exp
            )

            # ---- u = W_e^T v on TensorE, pipelined with chunked W DMA ----
            v_sb = consts.tile([P, HC], f32)
            nc.sync.dma_start(out=v_sb, in_=v.ap().rearrange("(c p) -> p c", p=P))
            w_r = w.ap().rearrange("(c p) d -> c p d", p=P)
            pu = [psum.tile([1, 512], f32, name=f"pu{i}") for i in range(2)]
            w_sb = []
            for c in range(HC):
                wc = wpool.tile([P, D], f32)
                nc.sync.dma_start(out=wc, in_=w_r[c])
                w_sb.append(wc)
            for c in range(HC):
                for half in range(2):
                    nc.tensor.matmul(
                        pu[half],
                        v_sb[:, c : c + 1],
                        w_sb[c][:, 512 * half : 512 * (half + 1)],
                        start=(c == 0),
                        stop=(c == HC - 1),
                    )
            u_sb = consts.tile([1, D], f32)
            for half in range(2):
                nc.vector.tensor_copy(
                    out=u_sb[:, 512 * half : 512 * (half + 1)], in_=pu[half]
                )
            u_bc = consts.tile([P, D], f32)
            nc.gpsimd.partition_broadcast(u_bc, u_sb)

            # ---- stream E, fused dot with u, per-batch softmax in SBUF ----
            e_r = e.ap().rearrange("(b p t) d -> b p t d", b=BL, p=P)
            out_r = out.ap().rearrange("b (p t) -> b p t", p=P)
            for b in range(BL):
                nrg = nrgs.tile([P, TB], f32)
                for t0 in range(0, TB, G):
                    ch = chunks.tile([P, G, D], f32)
                    nc.sync.dma_start(out=ch, in_=e_r[b, :, t0 : t0 + G, :])
                    for g in range(G):
                        # out = (e_tile * 1.0) * u ; accum_out = row-sum = e_row . u
                        # (tensor_tensor_reduce is broken on this runtime; this
                        # InstTensorScalarPtr form is the working fused dot.)
                        pr = prods.tile([P, D], f32)
                        nc.vector.scalar_tensor_tensor(
                            out=pr,
                            in0=ch[:, g, :],
                            scalar=1.0,
                            in1=u_bc,
                            op0=Alu.mult,
                            op1=Alu.mult,
                            accum_out=nrg[:, t0 + g : t0 + g + 1],
                        )

                # softmax over the 2048 energies of batch b ([128, TB] tile)
                mx = smax.tile([P, 1], f32)
                nc.vector.reduce_max(out=mx, in_=nrg, axis=mybir.AxisListType.X)
                am = smax.tile([P, 1], f32)
                nc.gpsimd.partition_all_reduce(am, mx, P, ReduceOp.max)
                nm = smax.tile([P, 1], f32)
                nc.vector.tensor_scalar_mul(out=nm, in0=am, scalar1=-1.0)
                prob = smax.tile([P, TB], f32)
                sums = smax.tile([P, 1], f32)
                nc.scalar.activation(
                    out=prob, in_=nrg, func=mybir.ActivationFunctionType.Exp,
                    bias=nm, scale=1.0, accum_out=sums,
                )
                gs = smax.tile([P, 1], f32)
                nc.gpsimd.partition_all_reduce(gs, sums, P, ReduceOp.add)
                rec = smax.tile([P, 1], f32)
                nc.vector.reciprocal(out=rec, in_=gs)
                res = smax.tile([P, TB], f32)
                nc.vector.tensor_scalar_mul(out=res, in0=prob, scalar1=rec)
                nc.sync.dma_start(out=out_r[b], in_=res)

    nc.compile()
    return nc


def kernel(encoder_outputs, rnn_hidden, W_attn, b_attn, v):
    global last_results
    from concourse.bass_utils import run_bass_kernel_spmd

    if "nc" not in _cache:
        _cache["nc"] = _build()
    nc = _cache["nc"]

    encoder_outputs = np.asarray(encoder_outputs, dtype=np.float32)
    w_e = np.ascontiguousarray(np.asarray(W_attn, dtype=np.float32)[:, H:])
    v_np = np.ascontiguousarray(np.asarray(v, dtype=np.float32))

    # Reorder rows so local row r = b*S + p*TB + t matches the kernel's
    # partition-major tiling: row (b, s) with s = p*TB + t keeps s intact,
    # so the natural [BL*S, D] flatten is already correct.
    in_maps = []
    for c in range(NCORES):
        e_c = np.ascontiguousarray(
            encoder_outputs[c * BL : (c + 1) * BL].reshape(BL * S, D)
        )
        in_maps.append({"e": e_c, "w": w_e, "v": v_np})

    last_results = run_bass_kernel_spmd(
        nc, in_maps, core_ids=list(range(NCORES)), trace=_PROFILE
    )
    outs = [last_results.results[c]["out"] for c in range(NCORES)]
    return np.concatenate(outs, axis=0).reshape(B, 1, S)
